# revision 22
# baseline (speedup 1.0000x reference)
"""Low-rank linear attention (causal, elu+1 feature map) on 8 trn2 cores.

Sharding: core = 2*b + h  (batch b in 0..3, sequence half h in 0..1).
Each core computes out[b, h*2048:(h+1)*2048, :].  Second-half cores
recompute the running K^T.V state over their 2048-token prefix on device
(sel scales the prefix contribution to zero on first-half cores so one
SPMD program serves all 8 cores).

v4 notes:
  - few fat DMA calls (multi-KB contiguous rows spread row-descriptors
    over all 16 queues; each dma_start costs ~0.5us of issue time).
  - all three projection passes stream rhs at full 128-row contraction;
    prefix runs token-major in fp8 e4m3 (x16 / W256 scaling) so no
    transposes are needed.
  - elu+1's "+1" lands in PSUM via ones-outer-product matmuls so
    evictions read PSUM directly.
  - every Phase B matmul is zero-padded to 128 contraction rows: the
    PE HAM clock gate watches array activity, and 64-row matmuls leave
    the clock throttled at 1.2 GHz.
  - 1/den folds into the output-projection evictions (per-partition
    scale on scalar + vector engines).

Shapes (hardcoded): B=4, S=4096, D=1024, K=64.  L = S/2 = 2048 tokens
per core, processed in 16 chunks of C=128.
"""

import numpy as np

B, S, D, K = 4, 4096, 1024, 64
L = S // 2          # tokens per core (main), also prefix length
C = 128             # chunk (tokens)
G = 512             # token group (4 chunks share one PSUM bank / evictions)
NCHUNK = L // C     # 16
NGRP = L // G       # 4
NDC = D // 128      # 8 contraction chunks
XS = 16.0           # prefix x fp8 scale
WS = 256.0          # prefix W fp8 scale
PSC = 1.0 / (XS * WS)

_cache = {}


def _build_nc():
    import concourse.bacc as bacc
    import concourse.tile as tile
    from concourse import mybir

    f32 = mybir.dt.float32
    bf16 = mybir.dt.bfloat16
    f8 = mybir.dt.float8e4
    AF = mybir.ActivationFunctionType
    Alu = mybir.AluOpType

    nc = bacc.Bacc()

    # x params are host-laid-out [p][d][c] so each DMA row is contiguous
    xm = nc.declare_dram_parameter("xm", [128, 8 * L], bf16, isOutput=False)
    xp8 = nc.declare_dram_parameter("xp8", [128, 8 * L], f8, isOutput=False)
    wqkm = nc.declare_dram_parameter("wqkm", [128, D + C], bf16,
                                     isOutput=False)
    wvk = nc.declare_dram_parameter("wvk", [128, D], bf16, isOutput=False)
    wkvp = nc.declare_dram_parameter("wkvp", [128, D], f8, isOutput=False)
    wot = nc.declare_dram_parameter("wot", [128, D], bf16, isOutput=False)
    sel = nc.declare_dram_parameter("sel", [C, 1], f32, isOutput=False)
    out = nc.declare_dram_parameter("out", [L, D], bf16, isOutput=True)

    with tile.TileContext(nc) as tc:
        with (
            tc.tile_pool(name="consts", bufs=1) as consts,
            tc.tile_pool(name="xmp", bufs=1) as xm_pool,
            tc.tile_pool(name="xpp", bufs=1) as xp_pool,
            tc.tile_pool(name="proj", bufs=1) as proj_pool,
            tc.tile_pool(name="vko", bufs=NGRP) as vko_pool,
            tc.tile_pool(name="vkop", bufs=NGRP) as vkop_pool,
            tc.tile_pool(name="small", bufs=6) as small,
            tc.tile_pool(name="tmp", bufs=6) as tmp_pool,
            tc.tile_pool(name="state_pool", bufs=1, space="PSUM") as state_pool,
        ):
            # ---- constants + x: few fat DMA calls, ordered so the first
            # group's operands land first ----
            wqkm_sb = consts.tile([128, D + C], bf16, tag="wqkm")
            wvk_sb3 = consts.tile([128, D], bf16, tag="wvk")
            wkvp_sb3 = consts.tile([128, D], f8, tag="wkvp")
            wot_sb = consts.tile([128, D], bf16, tag="wot")
            sel_sb = consts.tile([C, 1], f32, tag="sel")
            xm3 = xm_pool.tile([128, 8, L], bf16, tag="xm3")
            xp_sb = xp_pool.tile([128, 8, L], f8, tag="xp")
            xmv = xm[:, :].rearrange("p (d c) -> p d c", d=8)
            xpv = xp8[:, :].rearrange("p (d c) -> p d c", d=8)

            nc.sync.dma_start(out=wqkm_sb, in_=wqkm[:, :])
            nc.sync.dma_start(out=xm3[:, :, 0:G], in_=xmv[:, :, 0:G])
            nc.sync.dma_start(out=wvk_sb3, in_=wvk[:, :])
            nc.sync.dma_start(out=xm3[:, :, G:2 * G], in_=xmv[:, :, G:2 * G])
            nc.sync.dma_start(out=xp_sb[:, :, 0:D], in_=xpv[:, :, 0:D])
            nc.sync.dma_start(out=wkvp_sb3, in_=wkvp[:, :])
            nc.sync.dma_start(out=xp_sb[:, :, D:L], in_=xpv[:, :, D:L])
            nc.sync.dma_start(out=xm3[:, :, D:L], in_=xmv[:, :, D:L])
            nc.sync.dma_start(out=wot_sb, in_=wot[:, :])
            nc.sync.dma_start(out=sel_sb, in_=sel[:, :])

            wqk_sb = [wqkm_sb[:, d * 128:(d + 1) * 128] for d in range(NDC)]
            wvk_sb = [wvk_sb3[:, d * 128:(d + 1) * 128] for d in range(NDC)]
            wkvp_sb = [wkvp_sb3[:, d * 128:(d + 1) * 128] for d in range(NDC)]
            mask_sb = wqkm_sb[:, D:D + C]

            # on-device constant rows for the bias matmuls
            onesr = consts.tile([1, G], bf16, tag="onesr")
            nc.vector.memset(onesr, 1.0)
            vkb = consts.tile([1, 128], bf16, tag="vkb")
            nc.vector.memset(vkb[:, 0:K], 0.0)
            nc.vector.memset(vkb[:, K:128], 1.0)
            vkbp = consts.tile([1, 128], bf16, tag="vkbp")
            nc.vector.memset(vkbp[:, 0:K], 0.0)
            nc.vector.memset(vkbp[:, K:128], XS * WS)
            ones1 = consts.tile([1, 1], bf16, tag="ones1")
            nc.vector.memset(ones1, 1.0)
            onec_sb = consts.tile([C, 1], bf16, tag="onec")
            nc.vector.memset(onec_sb, 1.0)
            bm1 = consts.tile([128, 1], f32, tag="bm1")
            nc.vector.memset(bm1, -1.0)

            # persistent sbuf; q/k/ks/attn are zero-padded to 128 partitions
            # so every Phase B matmul contracts over the full PE array
            qT_sb = proj_pool.tile([128, L], bf16, tag="qT")
            kT_sb = proj_pool.tile([128, L], bf16, tag="kT")
            attn_all = proj_pool.tile([128, NCHUNK * C], bf16, tag="attn")
            nc.vector.memset(qT_sb[K:128, :], 0.0)
            nc.vector.memset(kT_sb[K:128, :], 0.0)
            nc.vector.memset(attn_all[K:128, :], 0.0)
            vkos = [vko_pool.tile([C, 4, 130], bf16, tag=f"vko{g}",
                                  name=f"vko{g}") for g in range(NGRP)]
            vkps = [vkop_pool.tile([C, 4, 130], bf16, tag=f"vkp{g}",
                                   name=f"vkp{g}") for g in range(NGRP)]
            kfss = [vkop_pool.tile([C, 4, K], bf16, tag=f"kfs{g}",
                                   name=f"kfs{g}") for g in range(NGRP)]
            ks_sb = small.tile([128, K + 1], bf16, tag="ks")
            nc.vector.memset(ks_sb[K:128, :], 0.0)

            # running state [K, K+1]: cols 0:K = S[k,m], col K = k_sum.
            state_ps = state_pool.tile([K, 1 + K], f32)

            def tok_major(xt, wt, bias_row, vko_g, g, dtype_note):
                """[V|1|K] token-major projection for one 4-chunk group."""
                pp = None
                for c4 in range(4):
                    sl = slice((g * 4 + c4) * C, (g * 4 + c4 + 1) * C)
                    if c4 == 0:
                        pp = pp_pool.tile([C, 4, 128], f32, tag="pp",
                                          name="pp")
                    for d in range(NDC):
                        nc.tensor.matmul(pp[:, c4, :], xt[:, d, sl], wt[d],
                                         start=(c4 == 0 and d == 0),
                                         stop=False, skip_group_check=True)
                    nc.tensor.matmul(pp[:, c4, :], onesr[:, 0:C], bias_row,
                                     start=False, stop=(c4 == 3),
                                     skip_group_check=True)
                return pp

            # =============== PHASE A+B interleaved ===============
            # Phase-1: group-0 main projections + the whole prefix (fp8
            # token-major) + prefix state.  Phase-2: remaining main
            # projections with attention chunks woven between them so the
            # PE stream never idles (the HAM clock gate throttles the PE
            # to 1.2 GHz after ~1us of idle and never recovers).
            with (
                tc.tile_pool(name="p1_ps", bufs=1, space="PSUM") as p1_pool,
                tc.tile_pool(name="pp_ps", bufs=1, space="PSUM") as pp_pool,
                tc.tile_pool(name="ostage", bufs=3) as ostage_pool,
            ):
                def qk_group(g):
                    gs = slice(g * G, (g + 1) * G)
                    p1 = p1_pool.tile([128, G], f32, tag="p1", name="p1")
                    for d in range(NDC):
                        nc.tensor.matmul(p1, wqk_sb[d], xm3[:, d, gs],
                                         start=(d == 0), stop=False)
                    nc.tensor.matmul(p1, ones1[:, 0:1].to_broadcast((1, 128)),
                                     onesr, start=False, stop=True)
                    e1 = tmp_pool.tile([128, G], f32, tag="e1", name="e1")
                    nc.scalar.activation(e1, p1, AF.Exp, bias=bm1)
                    nc.vector.scalar_tensor_tensor(
                        qT_sb[0:K, gs], e1[0:K, :], 1.0, p1[0:K, :],
                        op0=Alu.min, op1=Alu.max)
                    nc.vector.scalar_tensor_tensor(
                        kT_sb[0:K, gs], e1[K:2 * K, :], 1.0, p1[K:2 * K, :],
                        op0=Alu.min, op1=Alu.max)

                def kv_group(g):
                    pp = pp_pool.tile([C, 4, 128], f32, tag="pp", name="pp")
                    for c4 in range(4):
                        sl = slice((g * 4 + c4) * C, (g * 4 + c4 + 1) * C)
                        for d in range(NDC):
                            nc.tensor.matmul(pp[:, c4, :], xm3[:, d, sl],
                                             wvk_sb[d],
                                             start=(c4 == 0 and d == 0),
                                             stop=False,
                                             skip_group_check=True)
                        nc.tensor.matmul(pp[:, c4, :], onesr[:, 0:C], vkb,
                                         start=False, stop=(c4 == 3),
                                         skip_group_check=True)
                    vg = vkos[g]
                    nc.scalar.copy(vg[:, :, 0:K], pp[:, :, 0:K])
                    nc.vector.memset(vg[:, :, K:K + 1], 1.0)
                    e3 = tmp_pool.tile([C, 4, K], f32, tag="e3", name="e3")
                    nc.scalar.activation(e3, pp[:, :, K:128], AF.Exp,
                                         bias=bm1)
                    nc.vector.scalar_tensor_tensor(
                        vg[:, :, K + 1:2 * K + 1], e3, 1.0, pp[:, :, K:128],
                        op0=Alu.min, op1=Alu.max)

                # ---- Phase B helpers ----
                ats = [None] * NCHUNK
                atms = [None] * NCHUNK
                nds = [None] * NCHUNK
                dcs = [None] * NCHUNK
                recs = [None] * NCHUNK

                def vko_sl(i, a, b):
                    return vkos[i // 4][:, i % 4, a:b]

                an_pool = []
                op_pool = []

                def sc(i):
                    sl = slice(i * C, (i + 1) * C)
                    ats[i] = an_pool[0].tile([C, C], f32, tag="andc",
                                             name="at")
                    nc.tensor.matmul(ats[i], kT_sb[:, sl], qT_sb[:, sl],
                                     start=True, stop=True)

                def vecatm(i):
                    atms[i] = tmp_pool.tile([C, C], bf16, tag="atm",
                                            name="atm")
                    nc.vector.tensor_tensor(atms[i], ats[i], mask_sb, Alu.mult)

                def nd(i):
                    sl = slice(i * C, (i + 1) * C)
                    nds[i] = an_pool[0].tile([K, C], f32, tag="andc",
                                             name="nd")
                    nc.tensor.matmul(nds[i], vko_sl(i, 0, K), atms[i],
                                     start=True, stop=False)
                    nc.tensor.matmul(nds[i], ks_sb[:, 0:K], qT_sb[:, sl],
                                     start=False, stop=True)
                    dcs[i] = an_pool[0].tile([C, 1], f32, tag="andc",
                                             name="dc")
                    nc.tensor.matmul(dcs[i], atms[i], onec_sb,
                                     start=True, stop=False)
                    nc.tensor.matmul(dcs[i], qT_sb[:, sl], ks_sb[:, K:K + 1],
                                     start=False, stop=True)

                def st(i):
                    nc.tensor.matmul(state_ps, vko_sl(i, K + 1, 2 * K + 1),
                                     vko_sl(i, 0, K + 1),
                                     start=False, stop=(i == NCHUNK - 1),
                                     skip_group_check=True)

                def ksc(i):
                    if i < NCHUNK - 1:
                        nc.scalar.copy(ks_sb[0:K, :], state_ps)

                def recattn(i):
                    recs[i] = small.tile([C, 1], f32, tag="rec", name="rec")
                    nc.vector.reciprocal(recs[i], dcs[i])
                    nc.scalar.copy(attn_all[0:K, i * C:(i + 1) * C], nds[i])

                def op(i):
                    asl = attn_all[:, i * C:(i + 1) * C]
                    ost = ostage_pool.tile([C, D], bf16, tag="ost",
                                           name="ost")
                    o1 = op_pool[0].tile([C, D // 2], f32, tag="op",
                                         name="op")
                    nc.tensor.matmul(o1, asl, wot_sb[:, 0:512],
                                     start=True, stop=True)
                    o2 = op_pool[0].tile([C, D // 2], f32, tag="op",
                                         name="op")
                    nc.tensor.matmul(o2, asl, wot_sb[:, 512:1024],
                                     start=True, stop=True)
                    nc.scalar.activation(ost[:, 0:512], o1, AF.Copy,
                                         scale=recs[i])
                    nc.vector.tensor_scalar_mul(ost[:, 512:1024], o2,
                                                recs[i])
                    nc.sync.dma_start(out=out[i * C:(i + 1) * C, :], in_=ost)

                def emit_b(i):
                    nd(i)
                    st(i)
                    if i + 1 < NCHUNK:
                        sc(i + 1)
                    recattn(i)
                    if i + 1 < NCHUNK:
                        vecatm(i + 1)
                    ksc(i)
                    if i >= 1:
                        op(i - 1)

                # ---- phase-1 ----
                with tc.tile_pool(name="ppp_ps", bufs=2,
                                  space="PSUM") as ppp_pool:
                    def prefix_group(g):
                        ppx = ppp_pool.tile([C, 4, 128], f32, tag="ppp",
                                            name="ppx")
                        for c4 in range(4):
                            sl = slice((g * 4 + c4) * C,
                                       (g * 4 + c4 + 1) * C)
                            for d in range(NDC):
                                nc.tensor.matmul(ppx[:, c4, :],
                                                 xp_sb[:, d, sl],
                                                 wkvp_sb[d],
                                                 start=(c4 == 0 and d == 0),
                                                 stop=False,
                                                 skip_group_check=True)
                            nc.tensor.matmul(ppx[:, c4, :], onesr[:, 0:C],
                                             vkbp, start=False,
                                             stop=(c4 == 3),
                                             skip_group_check=True)
                        vp = vkps[g]
                        nc.scalar.mul(vp[:, :, 0:K], ppx[:, :, 0:K], PSC)
                        nc.vector.memset(vp[:, :, K:K + 1], 1.0)
                        e4 = tmp_pool.tile([C, 4, K], f32, tag="e4",
                                           name="e4")
                        nc.scalar.activation(e4, ppx[:, :, K:128], AF.Exp,
                                             scale=PSC, bias=bm1)
                        e4m = tmp_pool.tile([C, 4, K], f32, tag="e4m",
                                            name="e4m")
                        nc.vector.tensor_scalar_min(e4m, e4, 1.0)
                        nc.vector.scalar_tensor_tensor(
                            vp[:, :, K + 1:2 * K + 1], ppx[:, :, K:128], PSC,
                            e4m, op0=Alu.mult, op1=Alu.max)
                        nc.vector.tensor_scalar_mul(
                            kfss[g], vp[:, :, K + 1:2 * K + 1], sel_sb)

                    def prefix_states(g):
                        for c4 in range(4):
                            ci = g * 4 + c4
                            nc.tensor.matmul(
                                state_ps, kfss[g][:, c4, :],
                                vkps[g][:, c4, 0:K + 1],
                                start=(ci == 0), stop=False,
                                skip_group_check=True)

                    qk_group(0)
                    kv_group(0)
                    prefix_group(0)
                    prefix_group(1)
                    prefix_states(0)
                    qk_group(1)
                    prefix_group(2)
                    prefix_states(1)
                    kv_group(1)
                    prefix_group(3)
                    prefix_states(2)
                    prefix_states(3)
                    nc.scalar.copy(ks_sb[0:K, :], state_ps)

                # ---- phase-2: weave attention chunks between projection
                # groups so the PE never idles ----
                with (
                    tc.tile_pool(name="andc_ps", bufs=3,
                                 space="PSUM") as an_pool2,
                    tc.tile_pool(name="op_ps", bufs=2,
                                 space="PSUM") as op_pool2,
                ):
                    an_pool.append(an_pool2)
                    op_pool.append(op_pool2)
                    sc(0)
                    vecatm(0)
                    for i in range(4):
                        emit_b(i)
                    qk_group(2)
                    emit_b(4)
                    emit_b(5)
                    kv_group(2)
                    emit_b(6)
                    emit_b(7)
                    qk_group(3)
                    emit_b(8)
                    emit_b(9)
                    kv_group(3)
                    emit_b(10)
                    emit_b(11)
                    for i in range(12, NCHUNK):
                        emit_b(i)
                    op(NCHUNK - 1)

    nc.compile()
    worst = []
    for fn in nc.m.functions:
        for blk in fn.blocks:
            for inst in blk.instructions:
                n = len(inst.sync_info.on_wait) if inst.sync_info else 0
                if n > 1 and type(inst).__name__ == "InstMatmult":
                    worst.append((inst.name, n))
    if worst:
        import sys
        print(f"WARN: matmuls with >1 wait after lowering: {worst}",
              file=sys.stderr)
    return nc


def _prep_inputs(x, Wq, Wk, Wv, Wo):
    import ml_dtypes

    bf16 = ml_dtypes.bfloat16
    f8 = ml_dtypes.float8_e4m3

    def dmajor(Wcat):
        # [p, d*128+m] = Wcat[m, 128*d+p]
        return np.ascontiguousarray(
            Wcat.T.reshape(NDC, 128, 128).transpose(1, 0, 2).reshape(128, D))

    wqk = dmajor(np.concatenate([Wq, Wk], axis=0))           # [Q|K]
    wvkc = dmajor(np.concatenate([Wv, Wk], axis=0))          # [V|K]
    mask = np.triu(np.ones((C, C), np.float32))              # keep s <= t
    wqkm = np.concatenate([wqk, mask], axis=1).astype(bf16)
    wvk_h = wvkc.astype(bf16)
    wkvp = (WS * wvkc).astype(f8)                            # prefix, scaled
    wot = np.concatenate([Wo.T, np.zeros((D - K, D), np.float32)],
                         axis=0).astype(bf16)                # [128, D] padded
    zeros_xp = np.zeros((128, 8 * L), dtype=f8)

    def pdc(xt):
        # [D, L] -> [p][d][c] flat [128, 8*L]
        return np.ascontiguousarray(
            xt.reshape(NDC, 128, L).transpose(1, 0, 2).reshape(128, 8 * L))

    in_maps = []
    for core in range(8):
        b, h = core // 2, core % 2
        xb = x[b].T                                          # [D, S]
        if h:
            xp = pdc(XS * xb[:, 0:L]).astype(f8)
        else:
            xp = zeros_xp
        m = {
            "xm": pdc(xb[:, h * L:(h + 1) * L].astype(bf16)),
            "xp8": xp,
            "wqkm": wqkm,
            "wvk": wvk_h,
            "wkvp": wkvp,
            "wot": wot,
            "sel": np.full((C, 1), float(h), np.float32),
        }
        in_maps.append(m)
    return in_maps


def _run(inputs, trace=False):
    from concourse.bass_utils import run_bass_kernel_spmd

    if "nc" not in _cache:
        _cache["nc"] = _build_nc()
    nc = _cache["nc"]
    in_maps = _prep_inputs(
        np.asarray(inputs["x"], np.float32),
        np.asarray(inputs["Wq"], np.float32),
        np.asarray(inputs["Wk"], np.float32),
        np.asarray(inputs["Wv"], np.float32),
        np.asarray(inputs["Wo"], np.float32),
    )
    res = run_bass_kernel_spmd(nc, in_maps, list(range(8)), trace=trace)
    out = np.empty((B, S, D), np.float32)
    for core in range(8):
        b, h = core // 2, core % 2
        out[b, h * L:(h + 1) * L, :] = res.results[core]["out"].astype(
            np.float32)
    return out, res


def kernel(**inputs) -> np.ndarray:
    out, _ = _run(inputs, trace=False)
    return out


# revision 23
# speedup vs baseline: 1.0425x; 1.0425x over previous
"""Low-rank linear attention (causal, elu+1 feature map) on 8 trn2 cores.

Sharding: core = 2*b + h  (batch b in 0..3, sequence half h in 0..1).
Each core computes out[b, h*2048:(h+1)*2048, :].  Second-half cores
recompute the running K^T.V state over their 2048-token prefix on device
(sel scales the prefix contribution to zero on first-half cores so one
SPMD program serves all 8 cores).

v4 notes:
  - few fat DMA calls (multi-KB contiguous rows spread row-descriptors
    over all 16 queues; each dma_start costs ~0.5us of issue time).
  - all three projection passes stream rhs at full 128-row contraction;
    prefix runs token-major in fp8 e4m3 (x16 / W256 scaling) so no
    transposes are needed.
  - elu+1's "+1" lands in PSUM via ones-outer-product matmuls so
    evictions read PSUM directly.
  - every Phase B matmul is zero-padded to 128 contraction rows: the
    PE HAM clock gate watches array activity, and 64-row matmuls leave
    the clock throttled at 1.2 GHz.
  - 1/den folds into the output-projection evictions (per-partition
    scale on scalar + vector engines).

Shapes (hardcoded): B=4, S=4096, D=1024, K=64.  L = S/2 = 2048 tokens
per core, processed in 16 chunks of C=128.
"""

import numpy as np

B, S, D, K = 4, 4096, 1024, 64
L = S // 2          # tokens per core (main), also prefix length
C = 128             # chunk (tokens)
G = 512             # token group (4 chunks share one PSUM bank / evictions)
NCHUNK = L // C     # 16
NGRP = L // G       # 4
NDC = D // 128      # 8 contraction chunks
XS = 16.0           # prefix x fp8 scale
WS = 256.0          # prefix W fp8 scale
PSC = 1.0 / (XS * WS)

_cache = {}


def _build_nc():
    import concourse.bacc as bacc
    import concourse.tile as tile
    from concourse import mybir

    f32 = mybir.dt.float32
    bf16 = mybir.dt.bfloat16
    f8 = mybir.dt.float8e4
    AF = mybir.ActivationFunctionType
    Alu = mybir.AluOpType

    nc = bacc.Bacc()

    # x params are host-laid-out [p][d][c] so each DMA row is contiguous
    xm = nc.declare_dram_parameter("xm", [128, 8 * L], bf16, isOutput=False)
    xp8 = nc.declare_dram_parameter("xp8", [128, 8 * L], f8, isOutput=False)
    wqkm = nc.declare_dram_parameter("wqkm", [128, D + C], bf16,
                                     isOutput=False)
    wvk = nc.declare_dram_parameter("wvk", [128, D], bf16, isOutput=False)
    wkvp = nc.declare_dram_parameter("wkvp", [128, D], f8, isOutput=False)
    wot = nc.declare_dram_parameter("wot", [128, D], bf16, isOutput=False)
    sel = nc.declare_dram_parameter("sel", [C, 1], f32, isOutput=False)
    out = nc.declare_dram_parameter("out", [L, D], bf16, isOutput=True)

    with tile.TileContext(nc) as tc:
        with (
            tc.tile_pool(name="consts", bufs=1) as consts,
            tc.tile_pool(name="xmp", bufs=1) as xm_pool,
            tc.tile_pool(name="xpp", bufs=1) as xp_pool,
            tc.tile_pool(name="proj", bufs=1) as proj_pool,
            tc.tile_pool(name="vko", bufs=NGRP) as vko_pool,
            tc.tile_pool(name="vkop", bufs=NGRP) as vkop_pool,
            tc.tile_pool(name="small", bufs=6) as small,
            tc.tile_pool(name="tmp", bufs=6) as tmp_pool,
            tc.tile_pool(name="state_pool", bufs=1, space="PSUM") as state_pool,
        ):
            # ---- constants + x: few fat DMA calls, ordered so the first
            # group's operands land first ----
            wqkm_sb = consts.tile([128, D + C], bf16, tag="wqkm")
            wvk_sb3 = consts.tile([128, D], bf16, tag="wvk")
            wkvp_sb3 = consts.tile([128, D], f8, tag="wkvp")
            wot_sb = consts.tile([128, D], bf16, tag="wot")
            sel_sb = consts.tile([C, 1], f32, tag="sel")
            xm3 = xm_pool.tile([128, 8, L], bf16, tag="xm3")
            xp_sb = xp_pool.tile([128, 8, L], f8, tag="xp")
            xmv = xm[:, :].rearrange("p (d c) -> p d c", d=8)
            xpv = xp8[:, :].rearrange("p (d c) -> p d c", d=8)

            nc.sync.dma_start(out=wqkm_sb, in_=wqkm[:, :])
            nc.sync.dma_start(out=xm3[:, :, 0:G], in_=xmv[:, :, 0:G])
            nc.sync.dma_start(out=wvk_sb3, in_=wvk[:, :])
            nc.sync.dma_start(out=xm3[:, :, G:2 * G], in_=xmv[:, :, G:2 * G])
            nc.sync.dma_start(out=xp_sb[:, :, 0:D], in_=xpv[:, :, 0:D])
            nc.sync.dma_start(out=wkvp_sb3, in_=wkvp[:, :])
            nc.sync.dma_start(out=xp_sb[:, :, D:L], in_=xpv[:, :, D:L])
            nc.sync.dma_start(out=xm3[:, :, D:L], in_=xmv[:, :, D:L])
            nc.sync.dma_start(out=wot_sb, in_=wot[:, :])
            nc.sync.dma_start(out=sel_sb, in_=sel[:, :])

            wqk_sb = [wqkm_sb[:, d * 128:(d + 1) * 128] for d in range(NDC)]
            wvk_sb = [wvk_sb3[:, d * 128:(d + 1) * 128] for d in range(NDC)]
            wkvp_sb = [wkvp_sb3[:, d * 128:(d + 1) * 128] for d in range(NDC)]
            mask_sb = wqkm_sb[:, D:D + C]

            # on-device constant rows for the bias matmuls
            onesr = consts.tile([1, G], bf16, tag="onesr")
            nc.vector.memset(onesr, 1.0)
            vkb = consts.tile([1, 128], bf16, tag="vkb")
            nc.vector.memset(vkb[:, 0:K], 0.0)
            nc.vector.memset(vkb[:, K:128], 1.0)
            vkbp = consts.tile([1, 128], bf16, tag="vkbp")
            nc.vector.memset(vkbp[:, 0:K], 0.0)
            nc.vector.memset(vkbp[:, K:128], XS * WS)
            ones1 = consts.tile([1, 1], bf16, tag="ones1")
            nc.vector.memset(ones1, 1.0)
            onec_sb = consts.tile([C, 1], bf16, tag="onec")
            nc.vector.memset(onec_sb, 1.0)
            bm1 = consts.tile([128, 1], f32, tag="bm1")
            nc.vector.memset(bm1, -1.0)

            # persistent sbuf; q/k/ks/attn are zero-padded to 128 partitions
            # so every Phase B matmul contracts over the full PE array
            qT_sb = proj_pool.tile([128, L], bf16, tag="qT")
            kT_sb = proj_pool.tile([128, L], bf16, tag="kT")
            attn_all = proj_pool.tile([128, NCHUNK * C], bf16, tag="attn")
            nc.vector.memset(qT_sb[K:128, :], 0.0)
            nc.vector.memset(kT_sb[K:128, :], 0.0)
            nc.vector.memset(attn_all[K:128, :], 0.0)
            vkos = [vko_pool.tile([C, 4, 130], bf16, tag=f"vko{g}",
                                  name=f"vko{g}") for g in range(NGRP)]
            vkps = [vkop_pool.tile([C, 4, 130], bf16, tag=f"vkp{g}",
                                   name=f"vkp{g}") for g in range(NGRP)]
            kfss = [vkop_pool.tile([C, 4, K], bf16, tag=f"kfs{g}",
                                   name=f"kfs{g}") for g in range(NGRP)]
            ks_sb = small.tile([128, K + 1], bf16, tag="ks")
            nc.vector.memset(ks_sb[K:128, :], 0.0)

            # running state [K, K+1]: cols 0:K = S[k,m], col K = k_sum.
            state_ps = state_pool.tile([K, 1 + K], f32)

            def tok_major(xt, wt, bias_row, vko_g, g, dtype_note):
                """[V|1|K] token-major projection for one 4-chunk group."""
                pp = None
                for c4 in range(4):
                    sl = slice((g * 4 + c4) * C, (g * 4 + c4 + 1) * C)
                    if c4 == 0:
                        pp = pp_pool.tile([C, 4, 128], f32, tag="pp",
                                          name="pp")
                    for d in range(NDC):
                        nc.tensor.matmul(pp[:, c4, :], xt[:, d, sl], wt[d],
                                         start=(c4 == 0 and d == 0),
                                         stop=False, skip_group_check=True)
                    nc.tensor.matmul(pp[:, c4, :], onesr[:, 0:C], bias_row,
                                     start=False, stop=(c4 == 3),
                                     skip_group_check=True)
                return pp

            # =============== PHASE A+B interleaved ===============
            # Phase-1: group-0 main projections + the whole prefix (fp8
            # token-major) + prefix state.  Phase-2: remaining main
            # projections with attention chunks woven between them so the
            # PE stream never idles (the HAM clock gate throttles the PE
            # to 1.2 GHz after ~1us of idle and never recovers).
            with (
                tc.tile_pool(name="p1_ps", bufs=1, space="PSUM") as p1_pool,
                tc.tile_pool(name="pp_ps", bufs=1, space="PSUM") as pp_pool,
                tc.tile_pool(name="ostage", bufs=3) as ostage_pool,
            ):
                def qk_group(g):
                    gs = slice(g * G, (g + 1) * G)
                    p1 = p1_pool.tile([128, G], f32, tag="p1", name="p1")
                    for d in range(NDC):
                        nc.tensor.matmul(p1, wqk_sb[d], xm3[:, d, gs],
                                         start=(d == 0), stop=False)
                    nc.tensor.matmul(p1, ones1[:, 0:1].to_broadcast((1, 128)),
                                     onesr, start=False, stop=True)
                    e1 = tmp_pool.tile([128, G], f32, tag="e1", name="e1")
                    nc.scalar.activation(e1, p1, AF.Exp, bias=bm1)
                    nc.vector.scalar_tensor_tensor(
                        qT_sb[0:K, gs], e1[0:K, :], 1.0, p1[0:K, :],
                        op0=Alu.min, op1=Alu.max)
                    nc.vector.scalar_tensor_tensor(
                        kT_sb[0:K, gs], e1[K:2 * K, :], 1.0, p1[K:2 * K, :],
                        op0=Alu.min, op1=Alu.max)

                def kv_group(g):
                    pp = pp_pool.tile([C, 4, 128], f32, tag="pp", name="pp")
                    for c4 in range(4):
                        sl = slice((g * 4 + c4) * C, (g * 4 + c4 + 1) * C)
                        for d in range(NDC):
                            nc.tensor.matmul(pp[:, c4, :], xm3[:, d, sl],
                                             wvk_sb[d],
                                             start=(c4 == 0 and d == 0),
                                             stop=False,
                                             skip_group_check=True)
                        nc.tensor.matmul(pp[:, c4, :], onesr[:, 0:C], vkb,
                                         start=False, stop=(c4 == 3),
                                         skip_group_check=True)
                    vg = vkos[g]
                    nc.scalar.copy(vg[:, :, 0:K], pp[:, :, 0:K])
                    nc.vector.memset(vg[:, :, K:K + 1], 1.0)
                    e3 = tmp_pool.tile([C, 4, K], f32, tag="e3", name="e3")
                    nc.scalar.activation(e3, pp[:, :, K:128], AF.Exp,
                                         bias=bm1)
                    nc.vector.scalar_tensor_tensor(
                        vg[:, :, K + 1:2 * K + 1], e3, 1.0, pp[:, :, K:128],
                        op0=Alu.min, op1=Alu.max)

                # ---- Phase B helpers ----
                ats = [None] * NCHUNK
                atms = [None] * NCHUNK
                nds = [None] * NCHUNK
                dcs = [None] * NCHUNK
                recs = [None] * NCHUNK

                def vko_sl(i, a, b):
                    return vkos[i // 4][:, i % 4, a:b]

                an_pool = []
                op_pool = []

                def sc(i):
                    sl = slice(i * C, (i + 1) * C)
                    ats[i] = an_pool[0].tile([C, C], f32, tag="andc",
                                             name="at")
                    nc.tensor.matmul(ats[i], kT_sb[:, sl], qT_sb[:, sl],
                                     start=True, stop=True)

                def vecatm(i):
                    atms[i] = tmp_pool.tile([C, C], bf16, tag="atm",
                                            name="atm")
                    nc.vector.tensor_tensor(atms[i], ats[i], mask_sb, Alu.mult)

                def nd(i):
                    sl = slice(i * C, (i + 1) * C)
                    nds[i] = an_pool[0].tile([K, C], f32, tag="andc",
                                             name="nd")
                    nc.tensor.matmul(nds[i], vko_sl(i, 0, K), atms[i],
                                     start=True, stop=False)
                    nc.tensor.matmul(nds[i], ks_sb[:, 0:K], qT_sb[:, sl],
                                     start=False, stop=True)
                    dcs[i] = an_pool[0].tile([C, 1], f32, tag="andc",
                                             name="dc")
                    nc.tensor.matmul(dcs[i], atms[i], onec_sb,
                                     start=True, stop=False)
                    nc.tensor.matmul(dcs[i], qT_sb[:, sl], ks_sb[:, K:K + 1],
                                     start=False, stop=True)

                def st(i):
                    nc.tensor.matmul(state_ps, vko_sl(i, K + 1, 2 * K + 1),
                                     vko_sl(i, 0, K + 1),
                                     start=False, stop=(i == NCHUNK - 1),
                                     skip_group_check=True)

                def ksc(i):
                    if i < NCHUNK - 1:
                        nc.scalar.copy(ks_sb[0:K, :], state_ps)

                def recattn(i):
                    recs[i] = small.tile([C, 1], f32, tag="rec", name="rec")
                    nc.vector.reciprocal(recs[i], dcs[i])
                    nc.scalar.copy(attn_all[0:K, i * C:(i + 1) * C], nds[i])

                def op(i):
                    asl = attn_all[:, i * C:(i + 1) * C]
                    ost = ostage_pool.tile([C, D], bf16, tag="ost",
                                           name="ost")
                    o1 = op_pool[0].tile([C, D // 2], f32, tag="op",
                                         name="op")
                    nc.tensor.matmul(o1, asl, wot_sb[:, 0:512],
                                     start=True, stop=True)
                    o2 = op_pool[0].tile([C, D // 2], f32, tag="op",
                                         name="op")
                    nc.tensor.matmul(o2, asl, wot_sb[:, 512:1024],
                                     start=True, stop=True)
                    nc.scalar.activation(ost[:, 0:512], o1, AF.Copy,
                                         scale=recs[i])
                    nc.vector.tensor_scalar_mul(ost[:, 512:1024], o2,
                                                recs[i])
                    nc.sync.dma_start(out=out[i * C:(i + 1) * C, :], in_=ost)

                def emit_b(i):
                    nd(i)
                    st(i)
                    if i + 1 < NCHUNK:
                        sc(i + 1)
                    recattn(i)
                    if i + 1 < NCHUNK:
                        vecatm(i + 1)
                    ksc(i)
                    if i >= 1:
                        op(i - 1)

                # ---- phase-1 ----
                with tc.tile_pool(name="ppp_ps", bufs=2,
                                  space="PSUM") as ppp_pool:
                    qk_group(0)
                    kv_group(0)
                    for g in range(NGRP):
                        ppx = ppp_pool.tile([C, 4, 128], f32, tag="ppp",
                                            name="ppx")
                        for c4 in range(4):
                            sl = slice((g * 4 + c4) * C,
                                       (g * 4 + c4 + 1) * C)
                            for d in range(NDC):
                                nc.tensor.matmul(ppx[:, c4, :],
                                                 xp_sb[:, d, sl],
                                                 wkvp_sb[d],
                                                 start=(c4 == 0 and d == 0),
                                                 stop=False,
                                                 skip_group_check=True)
                            nc.tensor.matmul(ppx[:, c4, :], onesr[:, 0:C],
                                             vkbp, start=False,
                                             stop=(c4 == 3),
                                             skip_group_check=True)
                        vp = vkps[g]
                        nc.scalar.mul(vp[:, :, 0:K], ppx[:, :, 0:K], PSC)
                        nc.vector.memset(vp[:, :, K:K + 1], 1.0)
                        e4 = tmp_pool.tile([C, 4, K], f32, tag="e4",
                                           name="e4")
                        nc.scalar.activation(e4, ppx[:, :, K:128], AF.Exp,
                                             scale=PSC, bias=bm1)
                        e4m = tmp_pool.tile([C, 4, K], f32, tag="e4m",
                                            name="e4m")
                        nc.vector.tensor_scalar_min(e4m, e4, 1.0)
                        nc.vector.scalar_tensor_tensor(
                            vp[:, :, K + 1:2 * K + 1], ppx[:, :, K:128], PSC,
                            e4m, op0=Alu.mult, op1=Alu.max)
                        nc.vector.tensor_scalar_mul(
                            kfss[g], vp[:, :, K + 1:2 * K + 1], sel_sb)
                        if g > 0:
                            for c4 in range(4):
                                ci = (g - 1) * 4 + c4
                                nc.tensor.matmul(
                                    state_ps, kfss[g - 1][:, c4, :],
                                    vkps[g - 1][:, c4, 0:K + 1],
                                    start=(ci == 0), stop=False,
                                    skip_group_check=True)
                    for c4 in range(4):
                        nc.tensor.matmul(state_ps, kfss[NGRP - 1][:, c4, :],
                                         vkps[NGRP - 1][:, c4, 0:K + 1],
                                         start=False, stop=False,
                                         skip_group_check=True)
                    nc.scalar.copy(ks_sb[0:K, :], state_ps)

                # ---- phase-2: weave attention chunks between projection
                # groups so the PE never idles ----
                with (
                    tc.tile_pool(name="andc_ps", bufs=3,
                                 space="PSUM") as an_pool2,
                    tc.tile_pool(name="op_ps", bufs=2,
                                 space="PSUM") as op_pool2,
                ):
                    an_pool.append(an_pool2)
                    op_pool.append(op_pool2)
                    sc(0)
                    vecatm(0)
                    for g in range(1, NGRP):
                        qk_group(g)
                        emit_b(4 * (g - 1))
                        emit_b(4 * (g - 1) + 1)
                        kv_group(g)
                        emit_b(4 * (g - 1) + 2)
                        emit_b(4 * (g - 1) + 3)
                    for i in range(4 * (NGRP - 1), NCHUNK):
                        emit_b(i)
                    op(NCHUNK - 1)

    nc.compile()
    worst = []
    for fn in nc.m.functions:
        for blk in fn.blocks:
            for inst in blk.instructions:
                n = len(inst.sync_info.on_wait) if inst.sync_info else 0
                if n > 1 and type(inst).__name__ == "InstMatmult":
                    worst.append((inst.name, n))
    if worst:
        import sys
        print(f"WARN: matmuls with >1 wait after lowering: {worst}",
              file=sys.stderr)
    return nc


def _prep_inputs(x, Wq, Wk, Wv, Wo):
    import ml_dtypes

    bf16 = ml_dtypes.bfloat16
    f8 = ml_dtypes.float8_e4m3

    def dmajor(Wcat):
        # [p, d*128+m] = Wcat[m, 128*d+p]
        return np.ascontiguousarray(
            Wcat.T.reshape(NDC, 128, 128).transpose(1, 0, 2).reshape(128, D))

    wqk = dmajor(np.concatenate([Wq, Wk], axis=0))           # [Q|K]
    wvkc = dmajor(np.concatenate([Wv, Wk], axis=0))          # [V|K]
    mask = np.triu(np.ones((C, C), np.float32))              # keep s <= t
    wqkm = np.concatenate([wqk, mask], axis=1).astype(bf16)
    wvk_h = wvkc.astype(bf16)
    wkvp = (WS * wvkc).astype(f8)                            # prefix, scaled
    wot = np.concatenate([Wo.T, np.zeros((D - K, D), np.float32)],
                         axis=0).astype(bf16)                # [128, D] padded
    zeros_xp = np.zeros((128, 8 * L), dtype=f8)

    def pdc(xt):
        # [D, L] -> [p][d][c] flat [128, 8*L]
        return np.ascontiguousarray(
            xt.reshape(NDC, 128, L).transpose(1, 0, 2).reshape(128, 8 * L))

    in_maps = []
    for core in range(8):
        b, h = core // 2, core % 2
        xb = x[b].T                                          # [D, S]
        if h:
            xp = pdc(XS * xb[:, 0:L]).astype(f8)
        else:
            xp = zeros_xp
        m = {
            "xm": pdc(xb[:, h * L:(h + 1) * L].astype(bf16)),
            "xp8": xp,
            "wqkm": wqkm,
            "wvk": wvk_h,
            "wkvp": wkvp,
            "wot": wot,
            "sel": np.full((C, 1), float(h), np.float32),
        }
        in_maps.append(m)
    return in_maps


def _run(inputs, trace=False):
    from concourse.bass_utils import run_bass_kernel_spmd

    if "nc" not in _cache:
        _cache["nc"] = _build_nc()
    nc = _cache["nc"]
    in_maps = _prep_inputs(
        np.asarray(inputs["x"], np.float32),
        np.asarray(inputs["Wq"], np.float32),
        np.asarray(inputs["Wk"], np.float32),
        np.asarray(inputs["Wv"], np.float32),
        np.asarray(inputs["Wo"], np.float32),
    )
    res = run_bass_kernel_spmd(nc, in_maps, list(range(8)), trace=trace)
    out = np.empty((B, S, D), np.float32)
    for core in range(8):
        b, h = core // 2, core % 2
        out[b, h * L:(h + 1) * L, :] = res.results[core]["out"].astype(
            np.float32)
    return out, res


def kernel(**inputs) -> np.ndarray:
    out, _ = _run(inputs, trace=False)
    return out


# revision 24
# speedup vs baseline: 1.1892x; 1.1407x over previous
"""Low-rank linear attention (causal, elu+1 feature map) on 8 trn2 cores.

Sharding: core = 2*b + h  (batch b in 0..3, sequence half h in 0..1).
Each core computes out[b, h*2048:(h+1)*2048, :].  Second-half cores
recompute the running K^T.V state over their 2048-token prefix on device
(sel scales the prefix contribution to zero on first-half cores so one
SPMD program serves all 8 cores).

v4 notes:
  - few fat DMA calls (multi-KB contiguous rows spread row-descriptors
    over all 16 queues; each dma_start costs ~0.5us of issue time).
  - all three projection passes stream rhs at full 128-row contraction;
    prefix runs token-major in fp8 e4m3 (x16 / W256 scaling) so no
    transposes are needed.
  - elu+1's "+1" lands in PSUM via ones-outer-product matmuls so
    evictions read PSUM directly.
  - every Phase B matmul is zero-padded to 128 contraction rows: the
    PE HAM clock gate watches array activity, and 64-row matmuls leave
    the clock throttled at 1.2 GHz.
  - 1/den folds into the output-projection evictions (per-partition
    scale on scalar + vector engines).

Shapes (hardcoded): B=4, S=4096, D=1024, K=64.  L = S/2 = 2048 tokens
per core, processed in 16 chunks of C=128.
"""

import numpy as np

B, S, D, K = 4, 4096, 1024, 64
L = S // 2          # tokens per core (main), also prefix length
C = 128             # chunk (tokens)
G = 512             # token group (4 chunks share one PSUM bank / evictions)
NCHUNK = L // C     # 16
NGRP = L // G       # 4
NDC = D // 128      # 8 contraction chunks
XS = 16.0           # prefix x fp8 scale
WS = 256.0          # prefix W fp8 scale
PSC = 1.0 / (XS * WS)

_cache = {}


def _build_nc():
    import concourse.bacc as bacc
    import concourse.tile as tile
    from concourse import mybir

    f32 = mybir.dt.float32
    bf16 = mybir.dt.bfloat16
    f8 = mybir.dt.float8e4
    AF = mybir.ActivationFunctionType
    Alu = mybir.AluOpType

    nc = bacc.Bacc()

    # x params are host-laid-out [p][d][c] so each DMA row is contiguous
    xm = nc.declare_dram_parameter("xm", [128, 8 * L], bf16, isOutput=False)
    xp8 = nc.declare_dram_parameter("xp8", [128, 8 * L], f8, isOutput=False)
    wqkm = nc.declare_dram_parameter("wqkm", [128, D + C], bf16,
                                     isOutput=False)
    wvk = nc.declare_dram_parameter("wvk", [128, D], bf16, isOutput=False)
    wkvp = nc.declare_dram_parameter("wkvp", [128, D], f8, isOutput=False)
    wot = nc.declare_dram_parameter("wot", [128, D], bf16, isOutput=False)
    sel = nc.declare_dram_parameter("sel", [C, 1], f32, isOutput=False)
    out = nc.declare_dram_parameter("out", [L, D], bf16, isOutput=True)

    with tile.TileContext(nc) as tc:
        with (
            tc.tile_pool(name="consts", bufs=1) as consts,
            tc.tile_pool(name="xmp", bufs=1) as xm_pool,
            tc.tile_pool(name="xpp", bufs=1) as xp_pool,
            tc.tile_pool(name="proj", bufs=1) as proj_pool,
            tc.tile_pool(name="vko", bufs=NGRP) as vko_pool,
            tc.tile_pool(name="vkop", bufs=NGRP) as vkop_pool,
            tc.tile_pool(name="small", bufs=6) as small,
            tc.tile_pool(name="tmp", bufs=6) as tmp_pool,
            tc.tile_pool(name="state_pool", bufs=1, space="PSUM") as state_pool,
        ):
            # ---- constants + x: few fat DMA calls, ordered so the first
            # group's operands land first ----
            wqkm_sb = consts.tile([128, D + C], bf16, tag="wqkm")
            wvk_sb3 = consts.tile([128, D], bf16, tag="wvk")
            wkvp_sb3 = consts.tile([128, D], f8, tag="wkvp")
            wot_sb = consts.tile([128, D], bf16, tag="wot")
            sel_sb = consts.tile([C, 1], f32, tag="sel")
            xm3 = xm_pool.tile([128, 8, L], bf16, tag="xm3")
            xp_sb = xp_pool.tile([128, 8, L], f8, tag="xp")
            xmv = xm[:, :].rearrange("p (d c) -> p d c", d=8)
            xpv = xp8[:, :].rearrange("p (d c) -> p d c", d=8)

            nc.sync.dma_start(out=wqkm_sb, in_=wqkm[:, :])
            nc.sync.dma_start(out=xm3[:, :, 0:G], in_=xmv[:, :, 0:G])
            nc.sync.dma_start(out=wvk_sb3, in_=wvk[:, :])
            nc.sync.dma_start(out=xm3[:, :, G:2 * G], in_=xmv[:, :, G:2 * G])
            nc.sync.dma_start(out=xp_sb[:, :, 0:D], in_=xpv[:, :, 0:D])
            nc.sync.dma_start(out=wkvp_sb3, in_=wkvp[:, :])
            nc.sync.dma_start(out=xm3[:, :, D:L], in_=xmv[:, :, D:L])
            nc.sync.dma_start(out=xp_sb[:, :, D:L], in_=xpv[:, :, D:L])
            nc.sync.dma_start(out=wot_sb, in_=wot[:, :])
            nc.sync.dma_start(out=sel_sb, in_=sel[:, :])

            wqk_sb = [wqkm_sb[:, d * 128:(d + 1) * 128] for d in range(NDC)]
            wvk_sb = [wvk_sb3[:, d * 128:(d + 1) * 128] for d in range(NDC)]
            wkvp_sb = [wkvp_sb3[:, d * 128:(d + 1) * 128] for d in range(NDC)]
            mask_sb = wqkm_sb[:, D:D + C]

            # on-device constant rows for the bias matmuls
            onesr = consts.tile([1, G], bf16, tag="onesr")
            nc.vector.memset(onesr, 1.0)
            vkb = consts.tile([1, 128], bf16, tag="vkb")
            nc.vector.memset(vkb[:, 0:K], 0.0)
            nc.vector.memset(vkb[:, K:128], 1.0)
            vkbp = consts.tile([1, 128], bf16, tag="vkbp")
            nc.vector.memset(vkbp[:, 0:K], 0.0)
            nc.vector.memset(vkbp[:, K:128], XS * WS)
            ones1 = consts.tile([1, 1], bf16, tag="ones1")
            nc.vector.memset(ones1, 1.0)
            onec_sb = consts.tile([C, 1], bf16, tag="onec")
            nc.vector.memset(onec_sb, 1.0)
            bm1 = consts.tile([128, 1], f32, tag="bm1")
            nc.vector.memset(bm1, -1.0)

            # persistent sbuf; q/k/ks/attn are zero-padded to 128 partitions
            # so every Phase B matmul contracts over the full PE array
            qT_sb = proj_pool.tile([128, L], bf16, tag="qT")
            kT_sb = proj_pool.tile([128, L], bf16, tag="kT")
            attn_all = proj_pool.tile([128, NCHUNK * C], bf16, tag="attn")
            nc.vector.memset(qT_sb[K:128, :], 0.0)
            nc.vector.memset(kT_sb[K:128, :], 0.0)
            nc.vector.memset(attn_all[K:128, :], 0.0)
            vkos = [vko_pool.tile([C, 4, 130], bf16, tag=f"vko{g}",
                                  name=f"vko{g}") for g in range(NGRP)]
            vkps = [vkop_pool.tile([C, 4, 130], bf16, tag=f"vkp{g}",
                                   name=f"vkp{g}") for g in range(NGRP)]
            kfss = [vkop_pool.tile([C, 4, K], bf16, tag=f"kfs{g}",
                                   name=f"kfs{g}") for g in range(NGRP)]
            ks_sb = small.tile([128, K + 1], bf16, tag="ks")
            nc.vector.memset(ks_sb[K:128, :], 0.0)

            # running state [K, K+1]: cols 0:K = S[k,m], col K = k_sum.
            state_ps = state_pool.tile([K, 1 + K], f32)

            def tok_major(xt, wt, bias_row, vko_g, g, dtype_note):
                """[V|1|K] token-major projection for one 4-chunk group."""
                pp = None
                for c4 in range(4):
                    sl = slice((g * 4 + c4) * C, (g * 4 + c4 + 1) * C)
                    if c4 == 0:
                        pp = pp_pool.tile([C, 4, 128], f32, tag="pp",
                                          name="pp")
                    for d in range(NDC):
                        nc.tensor.matmul(pp[:, c4, :], xt[:, d, sl], wt[d],
                                         start=(c4 == 0 and d == 0),
                                         stop=False, skip_group_check=True)
                    nc.tensor.matmul(pp[:, c4, :], onesr[:, 0:C], bias_row,
                                     start=False, stop=(c4 == 3),
                                     skip_group_check=True)
                return pp

            # =============== PHASE A+B interleaved ===============
            # Phase-1: group-0 main projections + the whole prefix (fp8
            # token-major) + prefix state.  Phase-2: remaining main
            # projections with attention chunks woven between them so the
            # PE stream never idles (the HAM clock gate throttles the PE
            # to 1.2 GHz after ~1us of idle and never recovers).
            with (
                tc.tile_pool(name="p1_ps", bufs=1, space="PSUM") as p1_pool,
                tc.tile_pool(name="pp_ps", bufs=1, space="PSUM") as pp_pool,
                tc.tile_pool(name="ostage", bufs=3) as ostage_pool,
            ):
                def qk_group(g):
                    gs = slice(g * G, (g + 1) * G)
                    p1 = p1_pool.tile([128, G], f32, tag="p1", name="p1")
                    for d in range(NDC):
                        nc.tensor.matmul(p1, wqk_sb[d], xm3[:, d, gs],
                                         start=(d == 0), stop=False)
                    nc.tensor.matmul(p1, ones1[:, 0:1].to_broadcast((1, 128)),
                                     onesr, start=False, stop=True)
                    e1 = tmp_pool.tile([128, G], f32, tag="e1", name="e1")
                    nc.scalar.activation(e1, p1, AF.Exp, bias=bm1)
                    nc.vector.scalar_tensor_tensor(
                        qT_sb[0:K, gs], e1[0:K, :], 1.0, p1[0:K, :],
                        op0=Alu.min, op1=Alu.max)
                    nc.vector.scalar_tensor_tensor(
                        kT_sb[0:K, gs], e1[K:2 * K, :], 1.0, p1[K:2 * K, :],
                        op0=Alu.min, op1=Alu.max)

                def kv_group(g):
                    pp = pp_pool.tile([C, 4, 128], f32, tag="pp", name="pp")
                    for c4 in range(4):
                        sl = slice((g * 4 + c4) * C, (g * 4 + c4 + 1) * C)
                        for d in range(NDC):
                            nc.tensor.matmul(pp[:, c4, :], xm3[:, d, sl],
                                             wvk_sb[d],
                                             start=(c4 == 0 and d == 0),
                                             stop=False,
                                             skip_group_check=True)
                        nc.tensor.matmul(pp[:, c4, :], onesr[:, 0:C], vkb,
                                         start=False, stop=(c4 == 3),
                                         skip_group_check=True)
                    vg = vkos[g]
                    nc.scalar.copy(vg[:, :, 0:K], pp[:, :, 0:K])
                    nc.vector.memset(vg[:, :, K:K + 1], 1.0)
                    e3 = tmp_pool.tile([C, 4, K], f32, tag="e3", name="e3")
                    nc.scalar.activation(e3, pp[:, :, K:128], AF.Exp,
                                         bias=bm1)
                    nc.vector.scalar_tensor_tensor(
                        vg[:, :, K + 1:2 * K + 1], e3, 1.0, pp[:, :, K:128],
                        op0=Alu.min, op1=Alu.max)

                # ---- Phase B helpers ----
                ats = [None] * NCHUNK
                atms = [None] * NCHUNK
                nds = [None] * NCHUNK
                dcs = [None] * NCHUNK
                recs = [None] * NCHUNK

                def vko_sl(i, a, b):
                    return vkos[i // 4][:, i % 4, a:b]

                an_pool = []
                op_pool = []

                def sc(i):
                    sl = slice(i * C, (i + 1) * C)
                    ats[i] = an_pool[0].tile([C, C], f32, tag="andc",
                                             name="at")
                    nc.tensor.matmul(ats[i], kT_sb[:, sl], qT_sb[:, sl],
                                     start=True, stop=True)

                def vecatm(i):
                    atms[i] = tmp_pool.tile([C, C], bf16, tag="atm",
                                            name="atm")
                    nc.vector.tensor_tensor(atms[i], ats[i], mask_sb, Alu.mult)

                def nd(i):
                    sl = slice(i * C, (i + 1) * C)
                    nds[i] = an_pool[0].tile([K, C], f32, tag="andc",
                                             name="nd")
                    nc.tensor.matmul(nds[i], vko_sl(i, 0, K), atms[i],
                                     start=True, stop=False)
                    nc.tensor.matmul(nds[i], ks_sb[:, 0:K], qT_sb[:, sl],
                                     start=False, stop=True)
                    dcs[i] = an_pool[0].tile([C, 1], f32, tag="andc",
                                             name="dc")
                    nc.tensor.matmul(dcs[i], atms[i], onec_sb,
                                     start=True, stop=False)
                    nc.tensor.matmul(dcs[i], qT_sb[:, sl], ks_sb[:, K:K + 1],
                                     start=False, stop=True)

                def st(i):
                    nc.tensor.matmul(state_ps, vko_sl(i, K + 1, 2 * K + 1),
                                     vko_sl(i, 0, K + 1),
                                     start=False, stop=(i == NCHUNK - 1),
                                     skip_group_check=True)

                def ksc(i):
                    if i < NCHUNK - 1:
                        nc.scalar.copy(ks_sb[0:K, :], state_ps)

                def recattn(i):
                    recs[i] = small.tile([C, 1], f32, tag="rec", name="rec")
                    nc.vector.reciprocal(recs[i], dcs[i])
                    nc.scalar.copy(attn_all[0:K, i * C:(i + 1) * C], nds[i])

                def op(i):
                    asl = attn_all[:, i * C:(i + 1) * C]
                    ost = ostage_pool.tile([C, D], bf16, tag="ost",
                                           name="ost")
                    o1 = op_pool[0].tile([C, D // 2], f32, tag="op",
                                         name="op")
                    nc.tensor.matmul(o1, asl, wot_sb[:, 0:512],
                                     start=True, stop=True)
                    o2 = op_pool[0].tile([C, D // 2], f32, tag="op",
                                         name="op")
                    nc.tensor.matmul(o2, asl, wot_sb[:, 512:1024],
                                     start=True, stop=True)
                    nc.scalar.activation(ost[:, 0:512], o1, AF.Copy,
                                         scale=recs[i])
                    nc.vector.tensor_scalar_mul(ost[:, 512:1024], o2,
                                                recs[i])
                    nc.sync.dma_start(out=out[i * C:(i + 1) * C, :], in_=ost)

                def emit_b(i):
                    nd(i)
                    st(i)
                    if i + 1 < NCHUNK:
                        sc(i + 1)
                    recattn(i)
                    if i + 1 < NCHUNK:
                        vecatm(i + 1)
                    ksc(i)
                    if i >= 1:
                        op(i - 1)

                # ---- phase-1 ----
                with tc.tile_pool(name="ppp_ps", bufs=2,
                                  space="PSUM") as ppp_pool:
                    qk_group(0)
                    kv_group(0)
                    for g in range(NGRP):
                        ppx = ppp_pool.tile([C, 4, 128], f32, tag="ppp",
                                            name="ppx")
                        for c4 in range(4):
                            sl = slice((g * 4 + c4) * C,
                                       (g * 4 + c4 + 1) * C)
                            for d in range(NDC):
                                nc.tensor.matmul(ppx[:, c4, :],
                                                 xp_sb[:, d, sl],
                                                 wkvp_sb[d],
                                                 start=(c4 == 0 and d == 0),
                                                 stop=False,
                                                 skip_group_check=True)
                            nc.tensor.matmul(ppx[:, c4, :], onesr[:, 0:C],
                                             vkbp, start=False,
                                             stop=(c4 == 3),
                                             skip_group_check=True)
                        vp = vkps[g]
                        nc.scalar.mul(vp[:, :, 0:K], ppx[:, :, 0:K], PSC)
                        nc.vector.memset(vp[:, :, K:K + 1], 1.0)
                        e4 = tmp_pool.tile([C, 4, K], f32, tag="e4",
                                           name="e4")
                        nc.scalar.activation(e4, ppx[:, :, K:128], AF.Exp,
                                             scale=PSC, bias=bm1)
                        e4m = tmp_pool.tile([C, 4, K], f32, tag="e4m",
                                            name="e4m")
                        nc.vector.tensor_scalar_min(e4m, e4, 1.0)
                        nc.vector.scalar_tensor_tensor(
                            vp[:, :, K + 1:2 * K + 1], ppx[:, :, K:128], PSC,
                            e4m, op0=Alu.mult, op1=Alu.max)
                        nc.vector.tensor_scalar_mul(
                            kfss[g], vp[:, :, K + 1:2 * K + 1], sel_sb)
                        if g > 0:
                            for c4 in range(4):
                                ci = (g - 1) * 4 + c4
                                nc.tensor.matmul(
                                    state_ps, kfss[g - 1][:, c4, :],
                                    vkps[g - 1][:, c4, 0:K + 1],
                                    start=(ci == 0), stop=False,
                                    skip_group_check=True)
                    for c4 in range(4):
                        nc.tensor.matmul(state_ps, kfss[NGRP - 1][:, c4, :],
                                         vkps[NGRP - 1][:, c4, 0:K + 1],
                                         start=False, stop=False,
                                         skip_group_check=True)
                    nc.scalar.copy(ks_sb[0:K, :], state_ps)

                # ---- phase-2: weave attention chunks between projection
                # groups so the PE never idles ----
                with (
                    tc.tile_pool(name="andc_ps", bufs=3,
                                 space="PSUM") as an_pool2,
                    tc.tile_pool(name="op_ps", bufs=2,
                                 space="PSUM") as op_pool2,
                ):
                    an_pool.append(an_pool2)
                    op_pool.append(op_pool2)
                    sc(0)
                    vecatm(0)
                    for g in range(1, NGRP):
                        qk_group(g)
                        emit_b(4 * (g - 1))
                        emit_b(4 * (g - 1) + 1)
                        kv_group(g)
                        emit_b(4 * (g - 1) + 2)
                        emit_b(4 * (g - 1) + 3)
                    for i in range(4 * (NGRP - 1), NCHUNK):
                        emit_b(i)
                    op(NCHUNK - 1)

    nc.compile()
    worst = []
    for fn in nc.m.functions:
        for blk in fn.blocks:
            for inst in blk.instructions:
                n = len(inst.sync_info.on_wait) if inst.sync_info else 0
                if n > 1 and type(inst).__name__ == "InstMatmult":
                    worst.append((inst.name, n))
    if worst:
        import sys
        print(f"WARN: matmuls with >1 wait after lowering: {worst}",
              file=sys.stderr)
    return nc


def _prep_inputs(x, Wq, Wk, Wv, Wo):
    import ml_dtypes

    bf16 = ml_dtypes.bfloat16
    f8 = ml_dtypes.float8_e4m3

    def dmajor(Wcat):
        # [p, d*128+m] = Wcat[m, 128*d+p]
        return np.ascontiguousarray(
            Wcat.T.reshape(NDC, 128, 128).transpose(1, 0, 2).reshape(128, D))

    wqk = dmajor(np.concatenate([Wq, Wk], axis=0))           # [Q|K]
    wvkc = dmajor(np.concatenate([Wv, Wk], axis=0))          # [V|K]
    mask = np.triu(np.ones((C, C), np.float32))              # keep s <= t
    wqkm = np.concatenate([wqk, mask], axis=1).astype(bf16)
    wvk_h = wvkc.astype(bf16)
    wkvp = (WS * wvkc).astype(f8)                            # prefix, scaled
    wot = np.concatenate([Wo.T, np.zeros((D - K, D), np.float32)],
                         axis=0).astype(bf16)                # [128, D] padded
    zeros_xp = np.zeros((128, 8 * L), dtype=f8)

    def pdc(xt):
        # [D, L] -> [p][d][c] flat [128, 8*L]
        return np.ascontiguousarray(
            xt.reshape(NDC, 128, L).transpose(1, 0, 2).reshape(128, 8 * L))

    in_maps = []
    for core in range(8):
        b, h = core // 2, core % 2
        xb = x[b].T                                          # [D, S]
        if h:
            xp = pdc(XS * xb[:, 0:L]).astype(f8)
        else:
            xp = zeros_xp
        m = {
            "xm": pdc(xb[:, h * L:(h + 1) * L].astype(bf16)),
            "xp8": xp,
            "wqkm": wqkm,
            "wvk": wvk_h,
            "wkvp": wkvp,
            "wot": wot,
            "sel": np.full((C, 1), float(h), np.float32),
        }
        in_maps.append(m)
    return in_maps


def _run(inputs, trace=False):
    from concourse.bass_utils import run_bass_kernel_spmd

    if "nc" not in _cache:
        _cache["nc"] = _build_nc()
    nc = _cache["nc"]
    in_maps = _prep_inputs(
        np.asarray(inputs["x"], np.float32),
        np.asarray(inputs["Wq"], np.float32),
        np.asarray(inputs["Wk"], np.float32),
        np.asarray(inputs["Wv"], np.float32),
        np.asarray(inputs["Wo"], np.float32),
    )
    res = run_bass_kernel_spmd(nc, in_maps, list(range(8)), trace=trace)
    out = np.empty((B, S, D), np.float32)
    for core in range(8):
        b, h = core // 2, core % 2
        out[b, h * L:(h + 1) * L, :] = res.results[core]["out"].astype(
            np.float32)
    return out, res


def kernel(**inputs) -> np.ndarray:
    out, _ = _run(inputs, trace=False)
    return out
